# revision 1
# baseline (speedup 1.0000x reference)
"""Trainium2 Bass kernel for causal multi-head attention.

Shapes (hardcoded): B=4, T=2048, D=1024, H=16, Dh=64, fp32 I/O.

Strategy (8 NeuronCores, tensor-parallel over heads):
  - Each core c owns heads (2c, 2c+1): computes Q^T/K^T/V projections for its
    128 head-dims over the whole [B*T, D] input (contracting D on the PE),
    then causal flash-style attention in "scores-transposed" orientation
    (S^T[k, q] blocks) so softmax needs no on-chip transposes:
      * exp on ScalarE (no max subtraction: logits are O(+-4) by construction)
      * denominator via an appended ones-column in the V stationary operand
        (partition-axis reduction done by the PE itself)
      * division folded into the PSUM->SBUF copy against a PE-broadcast
        reciprocal
  - An on-device AllToAll re-shards ctx^T from head-sharded to row-sharded,
    then each core computes out rows [1024c : 1024c+1024) = ctx @ Wo + bo.
  - Host side only slices/cats and casts dtypes.

All matmul operands are fp16 (same PE throughput as bf16, 3 extra mantissa
bits); all accumulation is fp32 in PSUM.
"""

import sys

sys.path.insert(0, "/opt/trn_rl_repo")

import numpy as np

import concourse.bass as bass
import concourse.mybir as mybir
import concourse.tile as tile
from concourse import bacc
from concourse import bass_utils

N_CORES = 8
B, T, D, H, DH = 4, 2048, 1024, 16, 64
BT = B * T  # 8192
KS = D // 128  # 8 contraction subtiles
TC = 512  # t-chunk for projections
NTC = BT // TC  # 16
QC = 512  # query chunk in attention
NQC = T // QC  # 4 per batch
KB = 128  # key block
NKB = T // KB  # 16 per batch
ROWS = BT // N_CORES  # 1024 out rows per core

F16 = mybir.dt.float16
F32 = mybir.dt.float32

_CACHE = {}


def _build():
    nc = bacc.Bacc("TRN2", target_bir_lowering=False, num_devices=N_CORES)

    x_d = nc.dram_tensor("x", [D, BT], F16, kind="ExternalInput")  # pre-transposed
    wq_d = nc.dram_tensor("wq", [D, 128], F16, kind="ExternalInput")
    wk_d = nc.dram_tensor("wk", [D, 128], F16, kind="ExternalInput")
    wv_d = nc.dram_tensor("wv", [D, 128], F16, kind="ExternalInput")
    wo_d = nc.dram_tensor("wo", [D, D], F16, kind="ExternalInput")
    bo_d = nc.dram_tensor("bo", [D], F32, kind="ExternalInput")
    e2_d = nc.dram_tensor("e2", [2, 128], F16, kind="ExternalInput")
    cmask_d = nc.dram_tensor("cmask", [4, 128, QC], F16, kind="ExternalInput")
    out_d = nc.dram_tensor("out", [B, ROWS // B, D], F32, kind="ExternalOutput")

    with tile.TileContext(nc) as tc:
        with (
            tc.tile_pool(name="persist", bufs=1) as persist,
            tc.tile_pool(name="xt", bufs=4) as xtp,
            tc.tile_pool(name="work", bufs=12) as work,
            tc.tile_pool(name="tail", bufs=2) as tailp,
            tc.tile_pool(name="ctx", bufs=3) as ctxp,
            tc.tile_pool(name="outp", bufs=3) as outp,
            tc.tile_pool(name="ps_proj", bufs=1, space="PSUM") as ps_proj,
            tc.tile_pool(name="ps_s", bufs=3, space="PSUM") as ps_s,
            tc.tile_pool(name="ps_rb", bufs=1, space="PSUM") as ps_rb,
            tc.tile_pool(name="ps_av", bufs=3, space="PSUM") as ps_av,
            tc.tile_pool(name="dram", bufs=1, space="DRAM") as dram,
        ):
            # ---- persistent state ----
            wq_sb = persist.tile([128, KS, 128], F16)
            wk_sb = persist.tile([128, KS, 128], F16)
            wv_sb = persist.tile([128, KS, 128], F16)
            wo_sb = persist.tile([128, KS, D], F16)
            nc.sync.dma_start(wq_sb[:], wq_d.rearrange("(o p) h -> p o h", p=128))
            nc.sync.dma_start(wk_sb[:], wk_d.rearrange("(o p) h -> p o h", p=128))
            nc.sync.dma_start(wv_sb[:], wv_d.rearrange("(o p) h -> p o h", p=128))
            nc.sync.dma_start(wo_sb[:], wo_d.rearrange("(r p) n -> p r n", p=128))

            qt_sb = persist.tile([128, BT], F16)  # [2 heads x 64, global t]
            kt_sb = persist.tile([128, BT], F16)
            v0_sb = persist.tile([128, B * NKB, DH + 1], F16)  # + ones col
            v1_sb = persist.tile([128, B * NKB, DH + 1], F16)
            nc.vector.memset(v0_sb[:, :, DH : DH + 1], 1.0)
            nc.vector.memset(v1_sb[:, :, DH : DH + 1], 1.0)

            # bias broadcast [128, D] fp32 via PE ones-trick
            ones_col = persist.tile([1, 128], F32)
            nc.vector.memset(ones_col[:], 1.0)
            bo_sb = persist.tile([1, D], F32)
            nc.sync.dma_start(bo_sb[:], bo_d[None, :])
            bias_sb = persist.tile([128, D], F32)
            for nch in range(2):
                bps = ps_proj.tile([128, 512], F32, tag="proj")
                nc.tensor.matmul(
                    bps[:], ones_col[:], bo_sb[:, nch * 512 : (nch + 1) * 512]
                )
                nc.vector.tensor_copy(bias_sb[:, nch * 512 : (nch + 1) * 512], bps[:])

            # E2 selector for reciprocal broadcast: rows 0-63 <- r2[0], 64-127 <- r2[1]
            e2_sb = persist.tile([2, 128], F16)
            nc.sync.dma_start(e2_sb[:], e2_d[:])

            # diagonal causal masks: mask_i[p, j] = 1 if j >= p + i*128 else 0
            cmask4_sb = persist.tile([128, 4, QC], F16)
            nc.sync.dma_start(cmask4_sb[:], cmask_d.rearrange("i p j -> p i j"))

            # ---- phase 1: projections (emitted per batch, before that
            # batch's attention, so later batches' projections act as
            # low-priority PE gap-filler during attention) ----
            def emit_proj_chunk(tcn):
                t0 = tcn * TC
                xt = xtp.tile([128, KS, TC], F16, tag="xt", name="xt")
                nc.sync.dma_start(
                    xt[:],
                    x_d[:, t0 : t0 + TC].rearrange("(o p) t -> p o t", p=128),
                )
                for w_sb, dst in ((wq_sb, qt_sb), (wk_sb, kt_sb)):
                    pp = ps_proj.tile([128, TC], F32, tag="proj", name="pp")
                    for ks in range(KS):
                        nc.tensor.matmul(
                            pp[:],
                            w_sb[:, ks, :],
                            xt[:, ks, :],
                            start=(ks == 0),
                            stop=(ks == KS - 1),
                        )
                    nc.vector.tensor_copy(dst[:, t0 : t0 + TC], pp[:])
                for sub in range(TC // 128):
                    vp_full = ps_proj.tile([128, TC], F32, tag="proj", name="vp")
                    vp = vp_full[:, :128]
                    for ks in range(KS):
                        nc.tensor.matmul(
                            vp[:],
                            xt[:, ks, sub * 128 : (sub + 1) * 128],
                            wv_sb[:, ks, :],
                            start=(ks == 0),
                            stop=(ks == KS - 1),
                        )
                    kbg = tcn * (TC // 128) + sub
                    nc.vector.tensor_copy(v0_sb[:, kbg, 0:DH], vp[:, 0:DH])
                    nc.vector.tensor_copy(v1_sb[:, kbg, 0:DH], vp[:, DH : 2 * DH])

            # ---- phase 2: attention (scores-transposed flash) ----
            RB4 = ROWS // B  # 256 out rows per core per batch
            # last batch uses two half-batch collectives so its out-proj
            # overlaps the attention tail
            cc_ins = [dram.tile([N_CORES, 128, RB4], F16, name=f"cc_in{b}", tag=f"cc_in{b}") for b in range(B - 1)]
            cc_outs = [dram.tile([N_CORES, 128, RB4], F16, name=f"cc_out{b}", tag=f"cc_out{b}") for b in range(B - 1)]
            cc_ins_h = [dram.tile([N_CORES, 128, RB4 // 2], F16, name=f"cc_inh{i}", tag=f"cc_inh{i}") for i in range(2)]
            cc_outs_h = [dram.tile([N_CORES, 128, RB4 // 2], F16, name=f"cc_outh{i}", tag=f"cc_outh{i}") for i in range(2)]
            ao_sbs = []

            pending_oproj = []

            def emit_oproj_group(item):
                ob, oao, mb, nch = item
                t_in_ao = (mb * 128) % oao.shape[2]
                op = ps_proj.tile([128, 512], F32, tag="proj", name="op")
                for r in range(KS):
                    nc.tensor.matmul(
                        op[:],
                        oao[:, r, t_in_ao : t_in_ao + 128],
                        wo_sb[:, r, nch * 512 : (nch + 1) * 512],
                        start=(r == 0),
                        stop=(r == KS - 1),
                    )
                osb = outp.tile([128, 512], F32, tag="osb", name="osb")
                nc.vector.tensor_tensor(
                    osb[:], op[:], bias_sb[:, nch * 512 : (nch + 1) * 512],
                    mybir.AluOpType.add,
                )
                nc.sync.dma_start(
                    out_d[ob, mb * 128 : (mb + 1) * 128,
                          nch * 512 : (nch + 1) * 512],
                    osb[:],
                )

            for tcn in range(NTC):
                emit_proj_chunk(tcn)

            def emit_half_a2a(half):
                nc.gpsimd.collective_compute(
                    "AllToAll",
                    mybir.AluOpType.bypass,
                    replica_groups=[list(range(N_CORES))],
                    ins=[cc_ins_h[half][:]],
                    outs=[cc_outs_h[half][:]],
                )
                RBH = RB4 // 2
                ao_sb = persist.tile([128, KS, RBH], F16, name=f"aoh{half}", tag=f"aoh{half}")
                ao_sbs.append(ao_sb)
                nc.sync.dma_start(ao_sb[:], cc_outs_h[half].rearrange("r p t -> p r t"))
                for nch in range(2):
                    pending_oproj.append((B - 1, ao_sb, half, nch))

            for b in range(B):
                for qc in range(NQC):
                    if b == B - 1 and qc == 2:
                        emit_half_a2a(0)
                    for _ in range(4):
                        if pending_oproj:
                            emit_oproj_group(pending_oproj.pop(0))
                    q0 = b * T + qc * QC
                    nkb = 4 * qc + 4
                    av0_full = ps_av.tile([128, QC], F32, tag="av", name="av0")
                    av1_full = ps_av.tile([128, QC], F32, tag="av", name="av1")
                    av0 = av0_full[: DH + 1]
                    av1 = av1_full[: DH + 1]
                    def emit_se(kb):
                        # scores + exp (+ diag mask) for both heads of block kb
                        k0 = b * T + kb * KB
                        es = []
                        for h in (0, 1):
                            hs = slice(h * 64, (h + 1) * 64)
                            sp = ps_s.tile([128, QC], F32, tag="s", name="sp")
                            nc.tensor.matmul(
                                sp[:],
                                kt_sb[hs, k0 : k0 + QC // 4] if False else kt_sb[hs, k0 : k0 + KB],
                                qt_sb[hs, q0 : q0 + QC],
                            )
                            e = work.tile([128, QC], F16, tag="e", name="e")
                            nc.scalar.activation(
                                e[:], sp[:], mybir.ActivationFunctionType.Exp,
                                scale=0.125,
                            )
                            if kb >= 4 * qc:
                                nc.vector.tensor_tensor(
                                    e[:], e[:], cmask4_sb[:, kb - 4 * qc, :],
                                    mybir.AluOpType.mult,
                                )
                            es.append(e)
                        return es

                    e_next = emit_se(0)
                    for kb in range(nkb):
                        kbg = b * NKB + kb
                        first, last = kb == 0, kb == nkb - 1
                        e_cur = e_next
                        if not last:
                            e_next = emit_se(kb + 1)
                        for h, av in ((0, av0), (1, av1)):
                            vsb = v0_sb if h == 0 else v1_sb
                            nc.tensor.matmul(
                                av[:], vsb[:, kbg, :], e_cur[h][:],
                                start=first, stop=last,
                            )
                    # reciprocal of denominators (row 64 of each av bank)
                    u0 = tailp.tile([DH + 1, QC], F32, tag="u0")
                    u1 = tailp.tile([DH + 1, QC], F32, tag="u1")
                    nc.vector.tensor_copy(u0[:], av0[:])
                    nc.vector.tensor_copy(u1[:], av1[:])
                    d2a = tailp.tile([1, QC], F32, tag="d2a")
                    d2b = tailp.tile([1, QC], F32, tag="d2b")
                    nc.vector.tensor_copy(d2a[:], av0[DH : DH + 1, :])
                    nc.vector.tensor_copy(d2b[:], av1[DH : DH + 1, :])
                    r2 = tailp.tile([2, QC], F32, tag="r2")
                    r1t = tailp.tile([1, QC], F32, tag="r1t")
                    nc.vector.reciprocal_approx_fast(r2[0:1, :], d2a[:])
                    nc.vector.reciprocal_approx_fast(r1t[:], d2b[:])
                    nc.sync.dma_start(r2[1:2, :], r1t[:])
                    r2h = tailp.tile([2, QC], F16, tag="r2h")
                    nc.vector.tensor_copy(r2h[:], r2[:])
                    rb = ps_rb.tile([128, QC], F32, tag="rb")
                    nc.tensor.matmul(rb[:], e2_sb[:], r2h[:])
                    ctx2 = ctxp.tile([128, QC], F16, tag="ctx")
                    nc.vector.tensor_tensor(
                        ctx2[0:64, :], u0[0:64, :], rb[0:64, :],
                        mybir.AluOpType.mult,
                    )
                    nc.vector.tensor_tensor(
                        ctx2[64:128, :], u1[0:64, :], rb[64:128, :],
                        mybir.AluOpType.mult,
                    )
                    if b < B - 1:
                        s0 = qc * QC // RB4
                        nc.sync.dma_start(
                            cc_ins[b][s0 : s0 + QC // RB4].rearrange("s p f -> p s f"),
                            ctx2[:].rearrange("p (s f) -> p s f", s=QC // RB4),
                        )
                    else:
                        half, RBH = qc // 2, RB4 // 2
                        s0 = (qc % 2) * QC // RBH
                        nc.sync.dma_start(
                            cc_ins_h[half][s0 : s0 + QC // RBH].rearrange("s p f -> p s f"),
                            ctx2[:].rearrange("p (s f) -> p s f", s=QC // RBH),
                        )

                # ---- per-batch all-to-all; out-proj groups deferred ----
                if b < B - 1:
                    nc.gpsimd.collective_compute(
                        "AllToAll",
                        mybir.AluOpType.bypass,
                        replica_groups=[list(range(N_CORES))],
                        ins=[cc_ins[b][:]],
                        outs=[cc_outs[b][:]],
                    )
                    ao_sb = persist.tile([128, KS, RB4], F16, name=f"ao{b}", tag=f"ao{b}")
                    ao_sbs.append(ao_sb)
                    nc.sync.dma_start(ao_sb[:], cc_outs[b].rearrange("r p t -> p r t"))
                    for mb in range(RB4 // 128):
                        for nch in range(2):
                            pending_oproj.append((b, ao_sb, mb, nch))

            emit_half_a2a(1)
            while pending_oproj:
                emit_oproj_group(pending_oproj.pop(0))

    nc.compile()
    return nc


def _get_nc():
    if "nc" not in _CACHE:
        _CACHE["nc"] = _build()
    return _CACHE["nc"]


def prepare_in_maps(x, Wq, Wk, Wv, Wo, bo):
    x16 = np.ascontiguousarray(np.asarray(x, dtype=np.float32).reshape(BT, D).T).astype(np.float16)
    wo16 = np.asarray(Wo, dtype=np.float32).astype(np.float16)
    bo32 = np.ascontiguousarray(np.asarray(bo, dtype=np.float32))
    e2 = np.zeros((2, 128), dtype=np.float16)
    e2[0, 0:64] = 1.0
    e2[1, 64:128] = 1.0
    cmask = np.zeros((4, 128, QC), dtype=np.float16)
    for i in range(4):
        p = np.arange(128)[:, None]
        j = np.arange(QC)[None, :]
        cmask[i] = (j >= p + i * 128).astype(np.float16)
    in_maps = []
    for c in range(N_CORES):
        cs = slice(128 * c, 128 * (c + 1))
        in_maps.append(
            {
                "x": x16,
                "wq": np.ascontiguousarray(np.asarray(Wq, np.float32)[:, cs]).astype(np.float16),
                "wk": np.ascontiguousarray(np.asarray(Wk, np.float32)[:, cs]).astype(np.float16),
                "wv": np.ascontiguousarray(np.asarray(Wv, np.float32)[:, cs]).astype(np.float16),
                "wo": wo16,
                "bo": bo32,
                "e2": e2,
                "cmask": cmask,
            }
        )
    return in_maps


def kernel(x, Wq, Wk, Wv, Wo, bo, _trace=False):
    nc = _get_nc()
    in_maps = prepare_in_maps(x, Wq, Wk, Wv, Wo, bo)
    res = bass_utils.run_bass_kernel_spmd(
        nc, in_maps, list(range(N_CORES)), trace=_trace
    )
    if _trace:
        _CACHE["last_results"] = res
    out = np.empty((B, T, D), dtype=np.float32)
    rb4 = ROWS // B
    rbh = rb4 // 2
    for c in range(N_CORES):
        oc = res.results[c]["out"]  # [B, 256, D]
        for b in range(B - 1):
            out[b, rb4 * c : rb4 * (c + 1), :] = oc[b]
        # last batch was exchanged as two half-batch A2As with 128-row shards
        out[B - 1, rbh * c : rbh * (c + 1), :] = oc[B - 1, 0:rbh]
        out[B - 1, T // 2 + rbh * c : T // 2 + rbh * (c + 1), :] = oc[B - 1, rbh:]
    return out



# revision 22
# speedup vs baseline: 1.0024x; 1.0024x over previous
"""Trainium2 Bass kernel for causal multi-head attention.

Shapes (hardcoded): B=4, T=2048, D=1024, H=16, Dh=64, fp32 I/O.

Strategy (8 NeuronCores, tensor-parallel over heads):
  - Each core c owns heads (2c, 2c+1): Q^T/K^T projections for its 128
    head-dims over [B*T, D] (fp8e4m3 DoubleRow matmuls, contraction 256/step),
    V projection in fp16 (tokens-on-partitions layout for the AV stationary),
    then causal flash-style attention in scores-transposed orientation
    (S^T[k, q] blocks):
      * exp on ScalarE (no max subtraction: logits are O(+-3))
      * diagonal blocks emit only the unmasked column range (trim), with a
        single [128,128] triangle mask multiply
      * denominator via ones-columns embedded in the V stationary
      * division folded into PSUM->SBUF via a PE-broadcast reciprocal row
  - Per-(batch, query-chunk) AllToAll re-shards ctx^T from head-sharded to
    row-sharded (16 small A2As, fully pipelined); out-proj runs on 128-token
    pairs as PE gap-filler; out rows + bias DMA'd to DRAM.
  - Projection chunks for batch b+1 are woven between attention blocks of
    batch b so the PE never idles waiting on ScalarE exp.

All attention matmul operands are fp16; QK projections fp8e4m3 (DoubleRow);
accumulation fp32 in PSUM.
"""

import sys
from collections import deque

sys.path.insert(0, "/opt/trn_rl_repo")

import numpy as np

import concourse.bass as bass
import concourse.mybir as mybir
import concourse.tile as tile
from concourse import bacc
from concourse import bass_utils

N_CORES = 8
B, T, D, H, DH = 4, 2048, 1024, 16, 64
BT = B * T  # 8192
KS = D // 128  # 8 fp16 contraction subtiles
KS2 = D // 256  # 4 fp8 DoubleRow contraction subtiles
TC = 512  # t-chunk for projections
NTC = BT // TC  # 16
QC = 512  # query chunk in attention
NQC = T // QC  # 4 per batch
KB = 128  # key block
NKB = T // KB  # 16 per batch

F16 = mybir.dt.float16
F32 = mybir.dt.float32
F8 = mybir.dt.float8e4
DR = mybir.MatmulPerfMode.DoubleRow
EXP = mybir.ActivationFunctionType.Exp
MULT = mybir.AluOpType.mult
ADD = mybir.AluOpType.add

FP8_QK = True

_CACHE = {}


def _build(fp8_qk=FP8_QK):
    nc = bacc.Bacc("TRN2", target_bir_lowering=False, num_devices=N_CORES)

    if fp8_qk:
        x8_d = nc.dram_tensor("x8", [128, KS2, 2, BT], F8, kind="ExternalInput")
        wq_d = nc.dram_tensor("wq", [128, KS2, 2, 128], F8, kind="ExternalInput")
        wk_d = nc.dram_tensor("wk", [128, KS2, 2, 128], F8, kind="ExternalInput")
    else:
        wq_d = nc.dram_tensor("wq", [D, 128], F16, kind="ExternalInput")
        wk_d = nc.dram_tensor("wk", [D, 128], F16, kind="ExternalInput")
    x_d = nc.dram_tensor("x", [D, BT], F16, kind="ExternalInput")  # pre-transposed
    wv_d = nc.dram_tensor("wv", [D, 128], F16, kind="ExternalInput")
    wo_d = nc.dram_tensor("wo", [D, D], F16, kind="ExternalInput")
    bo_d = nc.dram_tensor("bo", [D], F32, kind="ExternalInput")
    tri_d = nc.dram_tensor("tri", [128, 128], F16, kind="ExternalInput")
    out_d = nc.dram_tensor("out", [B, NQC, 64, D], F32, kind="ExternalOutput")

    with tile.TileContext(nc) as tc:
        with (
            tc.tile_pool(name="persist", bufs=1) as persist,
            tc.tile_pool(name="x8p", bufs=2) as x8p,
            tc.tile_pool(name="x16p", bufs=2) as x16p,
            tc.tile_pool(name="work", bufs=8) as work,
            tc.tile_pool(name="tailp", bufs=2) as tailp,
            tc.tile_pool(name="ctxp", bufs=3) as ctxp,
            tc.tile_pool(name="outp", bufs=3) as outp,
            tc.tile_pool(name="ps", bufs=1, space="PSUM") as ps,
            tc.tile_pool(name="dram", bufs=1, space="DRAM") as dram,
        ):
            # ---- persistent weights / state ----
            if fp8_qk:
                wq_sb = persist.tile([128, KS2, 2, 128], F8)
                wk_sb = persist.tile([128, KS2, 2, 128], F8)
                nc.sync.dma_start(wq_sb[:], wq_d[:])
                nc.sync.dma_start(wk_sb[:], wk_d[:])
            else:
                wq_sb = persist.tile([128, KS, 128], F16)
                wk_sb = persist.tile([128, KS, 128], F16)
                nc.sync.dma_start(wq_sb[:], wq_d.rearrange("(o p) h -> p o h", p=128))
                nc.sync.dma_start(wk_sb[:], wk_d.rearrange("(o p) h -> p o h", p=128))
            wv_sb = persist.tile([128, KS, 128], F16)
            nc.sync.dma_start(wv_sb[:], wv_d.rearrange("(o p) h -> p o h", p=128))

            qt_sb = persist.tile([128, BT], F16)  # [2 heads x 64 dims, global t]
            kt_sb = persist.tile([128, BT], F16)
            # V layout per key-block kbg: per head a 65-col group [V 64 |
            # ones], so av[0:64] = ctx dims (partition range [0,64), aligned)
            # and av[64] = softmax denominator (partition base 64, aligned).
            v_sb = persist.tile([128, B * NKB, 130], F16)
            nc.vector.memset(v_sb[:, :, 64:65], 1.0)
            nc.vector.memset(v_sb[:, :, 129:130], 1.0)

            ones1 = persist.tile([1, 64], F16)
            nc.vector.memset(ones1[:], 1.0)
            tri_sb = persist.tile([128, 128], F16)
            nc.sync.dma_start(tri_sb[:], tri_d[:])
            ones_col = persist.tile([1, 128], F32)
            nc.vector.memset(ones_col[:], 1.0)
            bo_sb = persist.tile([1, D], F32)
            nc.sync.dma_start(bo_sb[:], bo_d[None, :])

            wo_sb = persist.tile([128, KS, D], F16)
            bias_sb = persist.tile([128, D], F32)
            ao_sbs = [
                persist.tile([128, KS, 256], F16, name=f"ao{b}", tag=f"ao{b}")
                for b in range(B)
            ]

            # ---- projection chunk units (PE gap-filler quanta) ----
            def chunk_units(tcn):
                t0 = tcn * TC
                st = {}

                def u_dma():
                    if fp8_qk:
                        st["x8"] = x8p.tile([128, KS2, 2, TC], F8, tag="x8", name="xt8")
                        nc.sync.dma_start(st["x8"][:], x8_d[:, :, :, t0 : t0 + TC])
                    st["x16"] = x16p.tile([128, KS, TC], F16, tag="x16", name="xt16")
                    nc.sync.dma_start(
                        st["x16"][:],
                        x_d[:, t0 : t0 + TC].rearrange("(o p) t -> p o t", p=128),
                    )

                def mk_qk(w_sb, dst):
                    def u():
                        pp = ps.tile([128, TC], F32, tag="pp", bufs=2, name="pp")
                        if fp8_qk:
                            for k in range(KS2):
                                nc.tensor.matmul(
                                    pp[:], w_sb[:, k], st["x8"][:, k],
                                    start=(k == 0), stop=(k == KS2 - 1),
                                    perf_mode=DR,
                                )
                        else:
                            for k in range(KS):
                                nc.tensor.matmul(
                                    pp[:], w_sb[:, k], st["x16"][:, k],
                                    start=(k == 0), stop=(k == KS - 1),
                                )
                        nc.vector.tensor_copy(dst[:, t0 : t0 + TC], pp[:])
                    return u

                def u_v():
                    # One self-contained unit: a pool tile's lifetime must be
                    # a contiguous emission window (other units allocate from
                    # the same tag slot rotation).
                    vp = ps.tile([128, 4, 128], F32, tag="pp", bufs=2, name="vp")
                    for sub in range(4):
                        for k in range(KS):
                            nc.tensor.matmul(
                                vp[:, sub, :],
                                st["x16"][:, k, sub * 128 : (sub + 1) * 128],
                                wv_sb[:, k],
                                start=(k == 0), stop=(k == KS - 1),
                            )
                    g0 = tcn * 4
                    nc.vector.tensor_copy(v_sb[:, g0 : g0 + 4, 0:64], vp[:, :, 0:64])
                    nc.vector.tensor_copy(
                        v_sb[:, g0 : g0 + 4, 65:129], vp[:, :, 64:128]
                    )

                return [u_dma, mk_qk(wq_sb, qt_sb), mk_qk(wk_sb, kt_sb), u_v]

            fillers = deque()

            def fill(n=1):
                for _ in range(n):
                    if fillers:
                        fillers.popleft()()

            # ---- out-proj tasks (also used as fillers) ----
            def mk_oproj(b, qc0, nch):
                def u():
                    ao = ao_sbs[b]
                    op = ps.tile([128, TC], F32, tag="pp", bufs=2, name="op")
                    for r in range(KS):
                        nc.tensor.matmul(
                            op[:],
                            ao[:, r, qc0 * 64 : (qc0 + 2) * 64],
                            wo_sb[:, r, nch * 512 : (nch + 1) * 512],
                            start=(r == 0), stop=(r == KS - 1),
                        )
                    osb = outp.tile([128, 512], F32, tag="osb", name="osb")
                    nc.vector.tensor_tensor(
                        osb[:], op[:], bias_sb[:, nch * 512 : (nch + 1) * 512], ADD
                    )
                    nc.sync.dma_start(
                        out_d[b, qc0 : qc0 + 2, :, nch * 512 : (nch + 1) * 512]
                        .rearrange("a p f -> (a p) f"),
                        osb[:],
                    )
                return u

            # ---- attention ----
            def emit_se(b, qc, kb):
                """Scores + exp (+ triangle mask on the diagonal 128-square)
                for both heads of key-block kb. Diagonal blocks emit only
                columns [128*i:] (i = kb - 4*qc)."""
                i = kb - 4 * qc
                off = 128 * i if i >= 0 else 0
                q0 = b * T + qc * QC
                k0 = b * T + kb * KB
                es = []
                for h in (0, 1):
                    hs = slice(h * 64, (h + 1) * 64)
                    sp = ps.tile([128, QC], F32, tag="s", bufs=3, name="sp")
                    nc.tensor.matmul(
                        sp[:, off:QC],
                        kt_sb[hs, k0 : k0 + KB],
                        qt_sb[hs, q0 + off : q0 + QC],
                    )
                    e = work.tile([128, QC], F16, tag="e", name="e")
                    nc.scalar.activation(e[:, off:QC], sp[:, off:QC], EXP, scale=0.125)
                    if i >= 0:
                        nc.vector.tensor_tensor(
                            e[:, off : off + 128], e[:, off : off + 128], tri_sb[:],
                            MULT,
                        )
                    es.append((e, off))
                return es

            def mk_tail(b, qc, av0, av1):
                def tail():
                    fill(1)
                    d2 = tailp.tile([1, 2 * QC], F32, tag="d2", name="d2")
                    nc.vector.tensor_copy(d2[:, 0:QC], av0[64:65, :])
                    nc.vector.tensor_copy(d2[:, QC : 2 * QC], av1[64:65, :])
                    r2 = tailp.tile([1, 2 * QC], F32, tag="r2", name="r2")
                    nc.vector.reciprocal_approx_fast(r2[:], d2[:])
                    r2h = tailp.tile([1, 2 * QC], F16, tag="r2h", name="r2h")
                    nc.vector.tensor_copy(r2h[:], r2[:])
                    fill(1)
                    rb = ps.tile([128, QC], F32, tag="s", bufs=3, name="rb")
                    nc.tensor.matmul(rb[0:64, :], ones1[:], r2h[:, 0:QC])
                    nc.tensor.matmul(rb[64:128, :], ones1[:], r2h[:, QC : 2 * QC])
                    rbsb = tailp.tile([128, QC], F32, tag="rbsb", name="rbsb")
                    nc.vector.tensor_copy(rbsb[:], rb[:])
                    ctx2 = ctxp.tile([128, QC], F16, tag="ctx", name="ctx")
                    nc.vector.tensor_tensor(
                        ctx2[0:64, :], av0[0:64, :], rbsb[0:64, :], MULT
                    )
                    nc.vector.tensor_tensor(
                        ctx2[64:128, :], av1[0:64, :], rbsb[64:128, :], MULT
                    )
                    cin = dram.tile(
                        [N_CORES, 128, 64], F16, tag=f"ci{b}_{qc}", name="ci"
                    )
                    cout = dram.tile(
                        [N_CORES, 128, 64], F16, tag=f"co{b}_{qc}", name="co"
                    )
                    nc.sync.dma_start(
                        cin[:].rearrange("s p f -> p s f"),
                        ctx2[:].rearrange("p (s f) -> p s f", s=N_CORES),
                    )
                    nc.gpsimd.collective_compute(
                        "AllToAll",
                        mybir.AluOpType.bypass,
                        replica_groups=[list(range(N_CORES))],
                        ins=[cin[:]],
                        outs=[cout[:]],
                    )
                    nc.sync.dma_start(
                        ao_sbs[b][:, :, qc * 64 : (qc + 1) * 64],
                        cout[:].rearrange("r p t -> p r t"),
                    )
                    if qc % 2 == 1:
                        for nch in range(2):
                            fillers.append(mk_oproj(b, qc - 1, nch))
                return tail

            # ---- emission ----
            # batch 0 projections upfront
            for tcn in range(4):
                for u in chunk_units(tcn):
                    u()
            # deferred big weight DMA + bias broadcast (PE ones-trick)
            nc.sync.dma_start(wo_sb[:], wo_d.rearrange("(r p) n -> p r n", p=128))
            for nchb in range(2):
                bps = ps.tile([128, 512], F32, tag="pp", bufs=2, name="bps")
                nc.tensor.matmul(
                    bps[:], ones_col[:], bo_sb[:, nchb * 512 : (nchb + 1) * 512]
                )
                nc.vector.tensor_copy(bias_sb[:, nchb * 512 : (nchb + 1) * 512], bps[:])

            prev_tail = None
            for b in range(B):
                if b + 1 < B:
                    for tcn in range(4 * (b + 1), 4 * (b + 2)):
                        fillers.extend(chunk_units(tcn))
                for qc in range(NQC):
                    nkb = 4 * qc + 4
                    window = deque(emit_se(b, qc, k) for k in range(2))
                    if prev_tail is not None:
                        prev_tail()
                    av0 = ps.tile([65, QC], F32, tag="av", bufs=3, name="av0")
                    av1 = ps.tile([65, QC], F32, tag="av", bufs=3, name="av1")
                    for kb in range(nkb):
                        if kb + 2 < nkb:
                            window.append(emit_se(b, qc, kb + 2))
                        fill(1)
                        (e0, off0), (e1, off1) = window.popleft()
                        kbg = b * NKB + kb
                        first, last = kb == 0, kb == nkb - 1
                        nc.tensor.matmul(
                            av0[:, off0:QC], v_sb[:, kbg, 0:65], e0[:, off0:QC],
                            start=first, stop=last,
                        )
                        nc.tensor.matmul(
                            av1[:, off1:QC], v_sb[:, kbg, 65:130], e1[:, off1:QC],
                            start=first, stop=last,
                        )
                    prev_tail = mk_tail(b, qc, av0, av1)
            prev_tail()
            while fillers:
                fillers.popleft()()

    nc.compile()
    return nc


def _get_nc():
    if "nc" not in _CACHE:
        _CACHE["nc"] = _build()
    return _CACHE["nc"]


def prepare_in_maps(x, Wq, Wk, Wv, Wo, bo):
    f8np = mybir.dt.np(F8)
    x32 = np.ascontiguousarray(np.asarray(x, dtype=np.float32).reshape(BT, D).T)
    x16 = x32.astype(np.float16)
    bo32 = np.ascontiguousarray(np.asarray(bo, dtype=np.float32))
    wo16 = np.asarray(Wo, dtype=np.float32).astype(np.float16)
    tri = (np.arange(QC // 4)[None, :] >= np.arange(128)[:, None]).astype(np.float16)

    in_maps = []
    if FP8_QK:
        x8 = np.ascontiguousarray(
            x32.reshape(KS2, 2, 128, BT).transpose(2, 0, 1, 3)
        ).astype(f8np)
    for c in range(N_CORES):
        cs = slice(128 * c, 128 * (c + 1))
        m = {
            "x": x16,
            "wv": np.ascontiguousarray(np.asarray(Wv, np.float32)[:, cs]).astype(
                np.float16
            ),
            "wo": wo16,
            "bo": bo32,
            "tri": tri,
        }
        if FP8_QK:
            m["x8"] = x8
            m["wq"] = np.ascontiguousarray(
                np.asarray(Wq, np.float32)[:, cs]
                .reshape(KS2, 2, 128, 128)
                .transpose(2, 0, 1, 3)
            ).astype(f8np)
            m["wk"] = np.ascontiguousarray(
                np.asarray(Wk, np.float32)[:, cs]
                .reshape(KS2, 2, 128, 128)
                .transpose(2, 0, 1, 3)
            ).astype(f8np)
        else:
            m["wq"] = np.ascontiguousarray(np.asarray(Wq, np.float32)[:, cs]).astype(
                np.float16
            )
            m["wk"] = np.ascontiguousarray(np.asarray(Wk, np.float32)[:, cs]).astype(
                np.float16
            )
        in_maps.append(m)
    return in_maps


def kernel(x, Wq, Wk, Wv, Wo, bo, _trace=False):
    nc = _get_nc()
    in_maps = prepare_in_maps(x, Wq, Wk, Wv, Wo, bo)
    res = bass_utils.run_bass_kernel_spmd(
        nc, in_maps, list(range(N_CORES)), trace=_trace
    )
    if _trace:
        _CACHE["last_results"] = res
    out = np.empty((B, T, D), dtype=np.float32)
    for c in range(N_CORES):
        oc = res.results[c]["out"]  # [B, NQC, 64, D]
        for qc in range(NQC):
            r0 = QC * qc + 64 * c
            out[:, r0 : r0 + 64, :] = oc[:, qc]
    return out
